# revision 8
# baseline (speedup 1.0000x reference)
"""Causal multi-head attention block (B=2, S=2048, M=1024, H=16, D=64) for 8
Trainium2 NeuronCores.

Sharding: tensor-parallel over heads (2 heads per core). Each core computes
QKV for its heads from the full x, runs causal attention, then an AllToAll
re-shards z so every core computes its 512-row slice of the output
projection against the full W_proj. Matmuls run in float32r (full PE rate,
~1e-4 relative error).

Self-contained: hardcodes all shapes; host-side numpy only shards/transposes
inputs and concatenates outputs.
"""

import numpy as np

import concourse.bass as bass
import concourse.bacc as bacc
import concourse.mybir as mybir
import concourse.tile as tile
from concourse.bass_utils import run_bass_kernel_spmd

B, S, M, H, D = 2, 2048, 1024, 16, 64
NC = 8
R = B * S                  # 4096 rows
HPC = H // NC              # 2 heads per core
MC = HPC * D               # 128 m-columns per core
P = 128
RB = 512                   # phase-1 row block
QB = 512                   # phase-2 query block
NRB = R // RB              # 8
NQB = S // QB              # 4 query blocks per batch
NMT = M // P               # 8 m-tiles
NVT = R // P               # 32 V row tiles
ROWS_PC = R // NC          # 512 output rows per core

f32 = mybir.dt.float32
f32r = mybir.dt.float32r
AF = mybir.ActivationFunctionType
ALU = mybir.AluOpType

_BUILD_CACHE = {}


def build_nc(with_bias=False):
    key = ("nc", with_bias)
    if key in _BUILD_CACHE:
        return _BUILD_CACHE[key]
    nc = bacc.Bacc("TRN2", target_bir_lowering=False, debug=False, num_devices=NC)

    xT = nc.dram_tensor("xT", [M, R], f32r, kind="ExternalInput").ap()
    wq = nc.dram_tensor("wq", [M, MC], f32r, kind="ExternalInput").ap()
    wk = nc.dram_tensor("wk", [M, MC], f32r, kind="ExternalInput").ap()
    wv = nc.dram_tensor("wv", [M, MC], f32r, kind="ExternalInput").ap()
    bqkv = nc.dram_tensor("bqkv", [P, 3], f32, kind="ExternalInput").ap()
    wp = nc.dram_tensor("wp", [M, M], f32r, kind="ExternalInput").ap()
    tri1 = nc.dram_tensor("tri1", [P, 128], f32r, kind="ExternalInput").ap()
    tri2 = nc.dram_tensor("tri2", [P, 256], f32r, kind="ExternalInput").ap()
    ident_d = nc.dram_tensor("ident_d", [P, P], f32r, kind="ExternalInput").ap()
    c65 = nc.dram_tensor("c65", [P, 65], f32r, kind="ExternalInput").ap()

    out = nc.dram_tensor("out", [ROWS_PC, M], f32, kind="ExternalOutput").ap()

    with tile.TileContext(nc) as tc:
        with (
            tc.tile_pool(name="cb", bufs=1) as cb,        # constants / persistents
            tc.tile_pool(name="dram", bufs=1, space="DRAM") as dram,
        ):
            # ---- constants ----
            wq_sb = cb.tile([P, NMT, MC], f32r)
            wk_sb = cb.tile([P, NMT, MC], f32r)
            wv_sb = cb.tile([P, NMT, MC], f32r)
            nc.sync.dma_start(wq_sb[:], wq.rearrange("(mt p) d -> p mt d", p=P))
            nc.sync.dma_start(wk_sb[:], wk.rearrange("(mt p) d -> p mt d", p=P))
            nc.sync.dma_start(wv_sb[:], wv.rearrange("(mt p) d -> p mt d", p=P))
            bias_sb = cb.tile([P, 3], f32)
            nc.sync.dma_start(bias_sb[:], bqkv[:])
            tri1_sb = cb.tile([P, 128], f32r)
            tri2_sb = cb.tile([P, 256], f32r)
            ident = cb.tile([P, P], f32r)
            c65_sb = cb.tile([P, 65], f32r)
            nc.sync.dma_start(tri1_sb[:], tri1[:])
            nc.sync.dma_start(tri2_sb[:], tri2[:])
            nc.sync.dma_start(ident[:], ident_d[:])
            nc.sync.dma_start(c65_sb[:], c65[:])

            # ---- persistent activations ----
            QT = cb.tile([P, R], f32r)        # [2h*64, rows], q pre-scaled
            KT = cb.tile([P, R], f32r)
            VA = cb.tile([P, NVT, 65], f32r)   # [V_A | ones]
            VB = cb.tile([P, NVT, P], f32r)    # [ones | 0*63 | V_B]
            ZT = cb.tile([P, R], f32r)

            # ================= phase 1: QKV =================
            with (
                tc.tile_pool(name="xp", bufs=2) as xp,
                tc.tile_pool(name="vp", bufs=2) as vp,
                tc.tile_pool(name="ps1", bufs=3, space="PSUM") as ps1,
            ):
                for rb in range(NRB):
                    r0 = rb * RB
                    xt = xp.tile([P, NMT, RB], f32r, tag="xt")
                    nc.sync.dma_start(
                        xt[:], xT[:, r0:r0 + RB].rearrange("(mt p) r -> p mt r", p=P))
                    for which, (w_sb, dst) in enumerate(
                            ((wq_sb, QT), (wk_sb, KT), (wv_sb, None))):
                        acc = ps1.tile([P, RB], f32, tag="qkv", name="acc")
                        for mt in range(NMT):
                            nc.tensor.matmul(acc[:], w_sb[:, mt, :], xt[:, mt, :],
                                             start=(mt == 0), stop=(mt == NMT - 1))
                        if with_bias:
                            bias_arg = dict(func=AF.Identity,
                                            bias=bias_sb[:, which:which + 1])
                        else:
                            bias_arg = dict(func=AF.Copy)
                        if dst is not None:
                            nc.scalar.activation(dst[:, r0:r0 + RB], acc[:],
                                                 **bias_arg)
                        else:
                            vt_sb = vp.tile([P, RB], f32r, tag="vt", name="vt_sb")
                            nc.scalar.activation(vt_sb[:], acc[:], **bias_arg)
                            for k in range(RB // P):
                                t = rb * (RB // P) + k
                                tp = ps1.tile([P, P], f32r, tag="tp", name="tp")
                                nc.tensor.transpose(tp[:], vt_sb[:, k * P:(k + 1) * P],
                                                    ident[:])
                                nc.vector.tensor_copy(VA[:, t, 0:64], tp[:, 0:64])
                                nc.vector.tensor_copy(VA[:, t, 64:65], c65_sb[:, 0:1])
                                nc.vector.tensor_copy(VB[:, t, 0:64], c65_sb[:, 0:64])
                                nc.vector.tensor_copy(VB[:, t, 64:128], tp[:, 64:128])

            # late pool for phase 3 weights (DMA starts early, overlaps phase 2)
            p3_cm = tc.tile_pool(name="p3", bufs=1)
            p3 = p3_cm.__enter__()
            wp_sb = p3.tile([P, NMT, M], f32r)
            nc.sync.dma_start(wp_sb[:], wp.rearrange("(mt p) n -> p mt n", p=P))

            # ================= phase 2: attention =================
            with (
                tc.tile_pool(name="ex", bufs=4) as exp_pool,
                tc.tile_pool(name="np_", bufs=2) as norm_pool,
                tc.tile_pool(name="ps2", bufs=1, space="PSUM") as ps2,
            ):
                for b in range(B):
                    for qb in range(NQB):
                        gr0 = b * S + qb * QB
                        zt_a = ps2.tile([65, QB], f32, tag="zta", bufs=2, name="zt_a")
                        zt_b = ps2.tile([P, QB], f32, tag="ztb", bufs=2, name="zt_b")
                        nkj = 4 * qb + 4
                        for t in range(nkj):
                            kj0 = 128 * t
                            di = t - 4 * qb
                            if di < 0:
                                col_off, w = 0, QB
                            elif di <= 1:
                                col_off, w = 128 * di, QB - 128 * di
                            else:
                                col_off, w = 256, 256
                            for h in range(2):
                                hp = slice(64 * h, 64 * h + 64)
                                st = ps2.tile([P, QB], f32, tag="st", bufs=4,
                                              name="st")
                                nc.tensor.matmul(
                                    st[:, :w],
                                    KT[hp, b * S + kj0: b * S + kj0 + 128],
                                    QT[hp, gr0 + col_off: gr0 + col_off + w],
                                    start=True, stop=True)
                                ex = exp_pool.tile([P, QB], f32r, tag="ex", name="ex")
                                nc.scalar.activation(ex[:, :w], st[:, :w], AF.Exp)
                                if di >= 0:
                                    if di == 3:
                                        nc.vector.tensor_tensor(
                                            ex[:, 0:256], ex[:, 0:256], tri2_sb[:],
                                            ALU.mult)
                                    else:
                                        nc.vector.tensor_tensor(
                                            ex[:, 0:128], ex[:, 0:128], tri1_sb[:],
                                            ALU.mult)
                                vt_idx = 16 * b + t
                                zt_x = zt_a if h == 0 else zt_b
                                lhsT = VA[:, vt_idx, :] if h == 0 else VB[:, vt_idx, :]
                                nc.tensor.matmul(
                                    zt_x[:, col_off:col_off + w], lhsT, ex[:, :w],
                                    start=(t == 0), stop=(t == nkj - 1),
                                    skip_group_check=True)
                        # normalize into ZT
                        recip = norm_pool.tile([P, QB], f32, tag="recip", name="recip")
                        nc.vector.reciprocal(recip[64:65, :], zt_a[64:65, :])
                        nc.vector.reciprocal(recip[0:1, :], zt_b[0:1, :])
                        rowa = norm_pool.tile([1, QB], f32, tag="rowa", name="rowa")
                        nc.sync.dma_start(rowa[:], recip[64:65, :])
                        bca = norm_pool.tile([64, QB], f32, tag="bca", name="bca")
                        bcb = norm_pool.tile([P, QB], f32, tag="bcb", name="bcb")
                        nc.gpsimd.partition_broadcast(bca[:], rowa[:], channels=64)
                        nc.gpsimd.partition_broadcast(bcb[:], recip[0:1, :],
                                                      channels=128)
                        nc.vector.tensor_tensor(ZT[0:64, gr0:gr0 + QB],
                                                zt_a[0:64, :], bca[:], ALU.mult)
                        nc.vector.tensor_tensor(ZT[64:128, gr0:gr0 + QB],
                                                zt_b[64:128, :], bcb[64:128, :],
                                                ALU.mult)

            # ================= all-to-all =================
            a2a_in = dram.tile([M, ROWS_PC], f32)
            a2a_out = dram.tile([M, ROWS_PC], f32)
            nc.sync.dma_start(
                a2a_in.rearrange("(j p) r -> p j r", p=P),
                ZT.bitcast(f32).rearrange("p (j r) -> p j r", r=ROWS_PC))
            nc.gpsimd.collective_compute(
                "AllToAll", ALU.bypass,
                replica_groups=[list(range(NC))],
                ins=[a2a_in.opt()], outs=[a2a_out.opt()],
            )

            # ================= phase 3: output projection =================
            zt_sb = p3.tile([P, NMT, ROWS_PC], f32r)
            nc.sync.dma_start(
                zt_sb[:],
                a2a_out.bitcast(f32r).rearrange("(mt p) r -> p mt r", p=P))
            with (
                tc.tile_pool(name="op", bufs=2) as out_pool,
                tc.tile_pool(name="ps3", bufs=4, space="PSUM") as ps3,
            ):
                for rt in range(ROWS_PC // P):
                    os_ = out_pool.tile([P, M], f32, tag="os", name="os_")
                    for nh in range(2):
                        acc = ps3.tile([P, 512], f32, tag="o", name="acc3")
                        for mt in range(NMT):
                            nc.tensor.matmul(
                                acc[:], zt_sb[:, mt, rt * P:(rt + 1) * P],
                                wp_sb[:, mt, nh * 512:(nh + 1) * 512],
                                start=(mt == 0), stop=(mt == NMT - 1))
                        nc.vector.tensor_copy(os_[:, nh * 512:(nh + 1) * 512], acc[:])
                    nc.sync.dma_start(out[rt * P:(rt + 1) * P, :], os_[:])
            p3_cm.__exit__(None, None, None)

    nc.compile()
    _BUILD_CACHE[key] = nc
    return nc


def prep_inputs(x, W_attn, b_attn, W_proj, b_proj):
    x = np.asarray(x, dtype=np.float32)
    W_attn = np.asarray(W_attn, dtype=np.float32)
    b_attn = np.asarray(b_attn, dtype=np.float32)
    W_proj = np.asarray(W_proj, dtype=np.float32)

    xT = np.ascontiguousarray(x.reshape(R, M).T)
    tri1 = (np.arange(128)[None, :] >= np.arange(128)[:, None]).astype(np.float32)
    tri2 = (np.arange(256)[None, :] >= (np.arange(128) + 128)[:, None]).astype(
        np.float32)
    ident = np.eye(P, dtype=np.float32)
    c65 = np.zeros((P, 65), dtype=np.float32)
    c65[:, 0] = 1.0
    scale = 1.0 / np.sqrt(D)

    in_maps = []
    for c in range(NC):
        cs = slice(MC * c, MC * (c + 1))
        bq = b_attn[0 * M:1 * M][cs] * scale
        bk = b_attn[1 * M:2 * M][cs]
        bv = b_attn[2 * M:3 * M][cs]
        in_maps.append({
            "xT": xT,
            "wq": np.ascontiguousarray(W_attn[:, 0 * M:1 * M][:, cs] * scale),
            "wk": np.ascontiguousarray(W_attn[:, 1 * M:2 * M][:, cs]),
            "wv": np.ascontiguousarray(W_attn[:, 2 * M:3 * M][:, cs]),
            "bqkv": np.ascontiguousarray(np.stack([bq, bk, bv], axis=1)),
            "wp": W_proj,
            "tri1": tri1, "tri2": tri2, "ident_d": ident, "c65": c65,
        })
    return in_maps


def postprocess(results, b_proj):
    out = np.concatenate([results[c]["out"] for c in range(NC)], axis=0)
    out = out + np.asarray(b_proj, dtype=np.float32)[None, :]
    return out.reshape(B, S, M)


def kernel(x, W_attn, b_attn, W_proj, b_proj):
    nc = build_nc(with_bias=bool(np.any(np.asarray(b_attn))))
    in_maps = prep_inputs(x, W_attn, b_attn, W_proj, b_proj)
    res = run_bass_kernel_spmd(nc, in_maps, core_ids=list(range(NC)))
    return postprocess(res.results, b_proj)


# revision 13
# speedup vs baseline: 1.7896x; 1.7896x over previous
"""Causal multi-head attention block (B=2, S=2048, M=1024, H=16, D=64) for 8
Trainium2 NeuronCores.

Sharding: tensor-parallel over heads (2 heads per core). Each core computes
QKV for its heads from the full x, runs causal attention, then an AllToAll
re-shards z so every core computes its 512-row slice of the output
projection against the full W_proj. Matmuls run in float32r (full PE rate,
~1e-4 relative error).

Self-contained: hardcodes all shapes; host-side numpy only shards/transposes
inputs and concatenates outputs.
"""

import numpy as np

import concourse.bass as bass
import concourse.bacc as bacc
import concourse.mybir as mybir
import concourse.tile as tile
from concourse.bass_utils import run_bass_kernel_spmd

B, S, M, H, D = 2, 2048, 1024, 16, 64
NC = 8
R = B * S                  # 4096 rows
HPC = H // NC              # 2 heads per core
MC = HPC * D               # 128 m-columns per core
P = 128
RB = 512                   # phase-1 row block
QB = 512                   # phase-2 query block
NRB = R // RB              # 8
NQB = S // QB              # 4 query blocks per batch
NMT = M // P               # 8 m-tiles
NVT = R // P               # 32 V row tiles
ROWS_PC = R // NC          # 512 output rows per core

f32 = mybir.dt.float32
f32r = mybir.dt.float32r
AF = mybir.ActivationFunctionType
ALU = mybir.AluOpType

_BUILD_CACHE = {}


def build_nc(with_bias=False, for_sim=False, phases=3):
    key = ("nc", with_bias, for_sim, phases)
    if key in _BUILD_CACHE:
        return _BUILD_CACHE[key]
    nc = bacc.Bacc("TRN2", target_bir_lowering=False, debug=False,
                   num_devices=1 if for_sim else NC)

    xT = nc.dram_tensor("xT", [M, R], f32r, kind="ExternalInput").ap()
    wq = nc.dram_tensor("wq", [M, MC], f32r, kind="ExternalInput").ap()
    wk = nc.dram_tensor("wk", [M, MC], f32r, kind="ExternalInput").ap()
    wv = nc.dram_tensor("wv", [M, MC], f32r, kind="ExternalInput").ap()
    bqkv = nc.dram_tensor("bqkv", [P, 3], f32, kind="ExternalInput").ap()
    wp = nc.dram_tensor("wp", [M, M], f32r, kind="ExternalInput").ap()
    tri1 = nc.dram_tensor("tri1", [P, 128], f32r, kind="ExternalInput").ap()
    tri2 = nc.dram_tensor("tri2", [P, 256], f32r, kind="ExternalInput").ap()
    ident_d = nc.dram_tensor("ident_d", [P, P], f32r, kind="ExternalInput").ap()
    c65 = nc.dram_tensor("c65", [P, 65], f32r, kind="ExternalInput").ap()

    out = nc.dram_tensor("out", [ROWS_PC, M], f32, kind="ExternalOutput").ap()

    with tile.TileContext(nc) as tc:
        with (
            tc.tile_pool(name="cb", bufs=1) as cb,        # constants / persistents
            tc.tile_pool(name="dram", bufs=1, space="DRAM") as dram,
        ):
            # ---- constants ----
            wq_sb = cb.tile([P, NMT, MC], f32r)
            wk_sb = cb.tile([P, NMT, MC], f32r)
            wv_sb = cb.tile([P, NMT, MC], f32r)
            nc.sync.dma_start(wq_sb[:], wq.rearrange("(mt p) d -> p mt d", p=P))
            nc.sync.dma_start(wk_sb[:], wk.rearrange("(mt p) d -> p mt d", p=P))
            nc.sync.dma_start(wv_sb[:], wv.rearrange("(mt p) d -> p mt d", p=P))
            bias_sb = cb.tile([P, 3], f32)
            nc.sync.dma_start(bias_sb[:], bqkv[:])
            tri1_sb = cb.tile([P, 128], f32r)
            tri2_sb = cb.tile([P, 256], f32r)
            ident = cb.tile([P, P], f32r)
            c65_sb = cb.tile([P, 65], f32r)
            nc.sync.dma_start(tri1_sb[:], tri1[:])
            nc.sync.dma_start(tri2_sb[:], tri2[:])
            nc.sync.dma_start(ident[:], ident_d[:])
            nc.sync.dma_start(c65_sb[:], c65[:])

            # ---- persistent activations ----
            QT = cb.tile([P, R], f32r)        # [2h*64, rows], q pre-scaled
            KT = cb.tile([P, R], f32r)
            VA = cb.tile([P, NVT, 65], f32r)   # [V_A | ones]
            VB = cb.tile([P, NVT, P], f32r)    # [ones | 0*63 | V_B]
            ZT = cb.tile([P, R], f32r)

            # ================= phase 1: QKV =================
            with (
                tc.tile_pool(name="xp", bufs=2) as xp,
                tc.tile_pool(name="vp", bufs=2) as vp,
                tc.tile_pool(name="ps1", bufs=3, space="PSUM") as ps1,
            ):
                for rb in range(NRB):
                    r0 = rb * RB
                    xt = xp.tile([P, NMT, RB], f32r, tag="xt")
                    nc.sync.dma_start(
                        xt[:], xT[:, r0:r0 + RB].rearrange("(mt p) r -> p mt r", p=P))
                    for which, (w_sb, dst) in enumerate(
                            ((wq_sb, QT), (wk_sb, KT), (wv_sb, None))):
                        acc = ps1.tile([P, RB], f32, tag="qkv", name="acc")
                        for mt in range(NMT):
                            nc.tensor.matmul(acc[:], w_sb[:, mt, :], xt[:, mt, :],
                                             start=(mt == 0), stop=(mt == NMT - 1))
                        if with_bias:
                            bias_arg = dict(func=AF.Identity,
                                            bias=bias_sb[:, which:which + 1])
                        else:
                            bias_arg = dict(func=AF.Copy)
                        if dst is not None:
                            nc.scalar.activation(dst[:, r0:r0 + RB], acc[:],
                                                 **bias_arg)
                        else:
                            vt_sb = vp.tile([P, RB], f32r, tag="vt", name="vt_sb")
                            nc.scalar.activation(vt_sb[:], acc[:], **bias_arg)
                            for k in range(RB // P):
                                t = rb * (RB // P) + k
                                tp = ps1.tile([P, P], f32r, tag="tp", name="tp")
                                nc.tensor.transpose(tp[:], vt_sb[:, k * P:(k + 1) * P],
                                                    ident[:])
                                nc.vector.tensor_copy(VA[:, t, 0:64], tp[:, 0:64])
                                nc.vector.tensor_copy(VA[:, t, 64:65], c65_sb[:, 0:1])
                                nc.vector.tensor_copy(VB[:, t, 0:64], c65_sb[:, 0:64])
                                nc.vector.tensor_copy(VB[:, t, 64:128], tp[:, 64:128])

            # late pool for phase 3 weights (DMA starts early, overlaps phase 2)
            p3_cm = tc.tile_pool(name="p3", bufs=1)
            p3 = p3_cm.__enter__()
            wp_sb = p3.tile([P, NMT, M], f32r)
            nc.sync.dma_start(wp_sb[:], wp.rearrange("(mt p) n -> p mt n", p=P))

            # ================= phase 2: attention =================
            with (
                tc.tile_pool(name="ex", bufs=4) as exp_pool,
                tc.tile_pool(name="np_", bufs=2) as norm_pool,
                tc.tile_pool(name="ps2", bufs=1, space="PSUM") as ps2,
            ):
                for b in range(B if phases >= 2 else 0):
                    for qb in range(NQB):
                        gr0 = b * S + qb * QB
                        zt_a = ps2.tile([65, QB], f32, tag="zta", bufs=2, name="zt_a")
                        zt_b = ps2.tile([P, QB], f32, tag="ztb", bufs=2, name="zt_b")
                        nkj = 4 * qb + 4
                        for t in range(nkj):
                            kj0 = 128 * t
                            di = t - 4 * qb
                            if di < 0:
                                col_off, w = 0, QB
                            elif di <= 1:
                                col_off, w = 128 * di, QB - 128 * di
                            else:
                                col_off, w = 256, 256
                            for h in range(2):
                                hp = slice(64 * h, 64 * h + 64)
                                st = ps2.tile([P, QB], f32, tag="st", bufs=4,
                                              name="st")
                                nc.tensor.matmul(
                                    st[:, :w],
                                    KT[hp, b * S + kj0: b * S + kj0 + 128],
                                    QT[hp, gr0 + col_off: gr0 + col_off + w],
                                    start=True, stop=True)
                                ex = exp_pool.tile([P, QB], f32r, tag="ex", name="ex")
                                nc.scalar.activation(ex[:, :w], st[:, :w], AF.Exp)
                                if di >= 0:
                                    if di == 3:
                                        nc.vector.tensor_tensor(
                                            ex[:, 0:256], ex[:, 0:256], tri2_sb[:],
                                            ALU.mult)
                                    else:
                                        nc.vector.tensor_tensor(
                                            ex[:, 0:128], ex[:, 0:128], tri1_sb[:],
                                            ALU.mult)
                                vt_idx = 16 * b + t
                                zt_x = zt_a if h == 0 else zt_b
                                lhsT = VA[:, vt_idx, :] if h == 0 else VB[:, vt_idx, :]
                                nc.tensor.matmul(
                                    zt_x[:, col_off:col_off + w], lhsT, ex[:, :w],
                                    start=(t == 0), stop=(t == nkj - 1),
                                    skip_group_check=True)
                        # normalize into ZT
                        recip = norm_pool.tile([P, QB], f32, tag="recip", name="recip")
                        nc.vector.reciprocal(recip[64:65, :], zt_a[64:65, :])
                        nc.vector.reciprocal(recip[0:1, :], zt_b[0:1, :])
                        rowa = norm_pool.tile([1, QB], f32, tag="rowa", name="rowa")
                        nc.sync.dma_start(rowa[:], recip[64:65, :])
                        bca = norm_pool.tile([64, QB], f32, tag="bca", name="bca")
                        bcb = norm_pool.tile([P, QB], f32, tag="bcb", name="bcb")
                        nc.gpsimd.partition_broadcast(bca[:], rowa[:], channels=64)
                        nc.gpsimd.partition_broadcast(bcb[:], recip[0:1, :],
                                                      channels=128)
                        nc.vector.tensor_tensor(ZT[0:64, gr0:gr0 + QB],
                                                zt_a[0:64, :], bca[:], ALU.mult)
                        nc.vector.tensor_tensor(ZT[64:128, gr0:gr0 + QB],
                                                zt_b[64:128, :], bcb[64:128, :],
                                                ALU.mult)

            # ================= all-to-all =================
            a2a_in = dram.tile([M, ROWS_PC], f32)
            a2a_out = dram.tile([M, ROWS_PC], f32)
            if phases >= 3:
                nc.sync.dma_start(
                    a2a_in.rearrange("(j p) r -> p j r", p=P),
                    ZT.bitcast(f32).rearrange("p (j r) -> p j r", r=ROWS_PC))
                if for_sim:
                    nc.sync.dma_start(a2a_out[:], a2a_in[:])
                else:
                    nc.gpsimd.collective_compute(
                        "AllToAll", ALU.bypass,
                        replica_groups=[list(range(NC))],
                        ins=[a2a_in.opt()], outs=[a2a_out.opt()],
                    )

            # ================= phase 3: output projection =================
            zt_sb = p3.tile([P, NMT, ROWS_PC], f32r)
            if phases >= 3:
                nc.sync.dma_start(
                    zt_sb[:],
                    a2a_out.bitcast(f32r).rearrange("(mt p) r -> p mt r", p=P))
            with (
                tc.tile_pool(name="op", bufs=2) as out_pool,
                tc.tile_pool(name="ps3", bufs=4, space="PSUM") as ps3,
            ):
                for rt in range(ROWS_PC // P if phases >= 3 else 0):
                    os_ = out_pool.tile([P, M], f32, tag="os", name="os_")
                    for nh in range(2):
                        acc = ps3.tile([P, 512], f32, tag="o", name="acc3")
                        for mt in range(NMT):
                            nc.tensor.matmul(
                                acc[:], zt_sb[:, mt, rt * P:(rt + 1) * P],
                                wp_sb[:, mt, nh * 512:(nh + 1) * 512],
                                start=(mt == 0), stop=(mt == NMT - 1))
                        nc.vector.tensor_copy(os_[:, nh * 512:(nh + 1) * 512], acc[:])
                    nc.sync.dma_start(out[rt * P:(rt + 1) * P, :], os_[:])
            p3_cm.__exit__(None, None, None)

    nc.compile()
    _BUILD_CACHE[key] = nc
    return nc


def prep_inputs(x, W_attn, b_attn, W_proj, b_proj):
    x = np.asarray(x, dtype=np.float32)
    W_attn = np.asarray(W_attn, dtype=np.float32)
    b_attn = np.asarray(b_attn, dtype=np.float32)
    W_proj = np.asarray(W_proj, dtype=np.float32)

    xT = np.ascontiguousarray(x.reshape(R, M).T)
    tri1 = (np.arange(128)[None, :] >= np.arange(128)[:, None]).astype(np.float32)
    tri2 = (np.arange(256)[None, :] >= (np.arange(128) + 128)[:, None]).astype(
        np.float32)
    ident = np.eye(P, dtype=np.float32)
    c65 = np.zeros((P, 65), dtype=np.float32)
    c65[:, 0] = 1.0
    scale = 1.0 / np.sqrt(D)

    in_maps = []
    for c in range(NC):
        cs = slice(MC * c, MC * (c + 1))
        bq = b_attn[0 * M:1 * M][cs] * scale
        bk = b_attn[1 * M:2 * M][cs]
        bv = b_attn[2 * M:3 * M][cs]
        in_maps.append({
            "xT": xT,
            "wq": np.ascontiguousarray(W_attn[:, 0 * M:1 * M][:, cs] * scale),
            "wk": np.ascontiguousarray(W_attn[:, 1 * M:2 * M][:, cs]),
            "wv": np.ascontiguousarray(W_attn[:, 2 * M:3 * M][:, cs]),
            "bqkv": np.ascontiguousarray(np.stack([bq, bk, bv], axis=1)),
            "wp": W_proj,
            "tri1": tri1, "tri2": tri2, "ident_d": ident, "c65": c65,
        })
    return in_maps


def postprocess(results, b_proj):
    out = np.concatenate([results[c]["out"] for c in range(NC)], axis=0)
    out = out + np.asarray(b_proj, dtype=np.float32)[None, :]
    return out.reshape(B, S, M)


def kernel(x, W_attn, b_attn, W_proj, b_proj):
    nc = build_nc(with_bias=bool(np.any(np.asarray(b_attn))))
    in_maps = prep_inputs(x, W_attn, b_attn, W_proj, b_proj)
    res = run_bass_kernel_spmd(nc, in_maps, core_ids=list(range(NC)))
    return postprocess(res.results, b_proj)


# revision 30
# speedup vs baseline: 30.3036x; 16.9335x over previous
"""Causal multi-head attention block (B=2, S=2048, M=1024, H=16, D=64) for 8
Trainium2 NeuronCores.

Sharding: tensor-parallel over heads (2 heads per core). Each core computes
QKV for its heads from the full x, runs causal attention, then an AllToAll
re-shards z so every core computes its 512-row slice of the output
projection against the full W_proj. Matmuls run in float32r (full PE rate,
~1e-4 relative error). Phase-1 (QKV) and phase-2 (attention) emission is
interleaved so ACT exp work overlaps PE matmul work.

Self-contained: hardcodes all shapes; host-side numpy only shards/transposes
inputs and concatenates outputs.
"""

import numpy as np

import concourse.bass as bass
import concourse.bacc as bacc
import concourse.mybir as mybir
import concourse.tile as tile
from concourse.bass_utils import run_bass_kernel_spmd

B, S, M, H, D = 2, 2048, 1024, 16, 64
NC = 8
R = B * S                  # 4096 rows
HPC = H // NC              # 2 heads per core
MC = HPC * D               # 128 m-columns per core
P = 128
RB = 512                   # phase-1 row block
QB = 512                   # phase-2 query block
NRB = R // RB              # 8
NQB = S // QB              # 4 query blocks per batch
NMT = M // P               # 8 m-tiles
NVT = R // P               # 32 V row tiles
ROWS_PC = R // NC          # 512 output rows per core

f32 = mybir.dt.float32
f32r = mybir.dt.float32r
AF = mybir.ActivationFunctionType
ALU = mybir.AluOpType

_BUILD_CACHE = {}

TUNE = {"acc_bufs": 2, "tp_own": False, "tp_bufs": 2, "st2_bufs": 2,
        "ex_bufs": 4, "xp_bufs": 2, "wp_late": True}


def build_nc(with_bias=False, for_sim=False, phases=3, repeat=1):
    key = ("nc", with_bias, for_sim, phases, repeat,
           tuple(sorted(TUNE.items())))
    if key in _BUILD_CACHE:
        return _BUILD_CACHE[key]
    nc = bacc.Bacc("TRN2", target_bir_lowering=False, debug=False,
                   num_devices=1 if for_sim else NC)

    xT = nc.dram_tensor("xT", [M, R], f32r, kind="ExternalInput").ap()
    wq = nc.dram_tensor("wq", [M, MC], f32r, kind="ExternalInput").ap()
    wk = nc.dram_tensor("wk", [M, MC], f32r, kind="ExternalInput").ap()
    wv = nc.dram_tensor("wv", [M, MC], f32r, kind="ExternalInput").ap()
    bqkv = nc.dram_tensor("bqkv", [P, 3], f32, kind="ExternalInput").ap()
    wp = nc.dram_tensor("wp", [M, M], f32r, kind="ExternalInput").ap()
    tri1 = nc.dram_tensor("tri1", [P, 128], f32r, kind="ExternalInput").ap()
    tri2 = nc.dram_tensor("tri2", [P, 256], f32r, kind="ExternalInput").ap()
    ident_d = nc.dram_tensor("ident_d", [P, P], f32r, kind="ExternalInput").ap()
    c65 = nc.dram_tensor("c65", [P, 65], f32r, kind="ExternalInput").ap()

    out = nc.dram_tensor("out", [ROWS_PC, M], f32, kind="ExternalOutput").ap()

    with tile.TileContext(nc) as tc:
        with (
            tc.tile_pool(name="cb", bufs=1) as cb,        # constants / persistents
            tc.tile_pool(name="dram", bufs=1, space="DRAM") as dram,
        ):
            # ---- constants ----
            wq_sb = cb.tile([P, NMT, MC], f32r)
            wk_sb = cb.tile([P, NMT, MC], f32r)
            wv_sb = cb.tile([P, NMT, MC], f32r)
            nc.sync.dma_start(wq_sb[:], wq.rearrange("(mt p) d -> p mt d", p=P))
            nc.sync.dma_start(wk_sb[:], wk.rearrange("(mt p) d -> p mt d", p=P))
            nc.sync.dma_start(wv_sb[:], wv.rearrange("(mt p) d -> p mt d", p=P))
            bias_sb = cb.tile([P, 3], f32)
            nc.sync.dma_start(bias_sb[:], bqkv[:])
            tri1_sb = cb.tile([P, 128], f32r)
            tri2_sb = cb.tile([P, 256], f32r)
            ident = cb.tile([P, P], f32r)
            c65_sb = cb.tile([P, 65], f32r)
            nc.sync.dma_start(tri1_sb[:], tri1[:])
            nc.sync.dma_start(tri2_sb[:], tri2[:])
            nc.sync.dma_start(ident[:], ident_d[:])
            nc.sync.dma_start(c65_sb[:], c65[:])

            # ---- persistent activations ----
            QT = cb.tile([P, R], f32r)        # [2h*64, rows], q pre-scaled
            KT = cb.tile([P, R], f32r)
            VA = cb.tile([P, NVT, 65], f32r)   # [V_A | ones]
            VB = cb.tile([P, NVT, P], f32r)    # [ones | 0*63 | V_B]
            ZT = cb.tile([P, R], f32r)

            # phase-3 weights
            wp_sb = cb.tile([P, NMT, M], f32r)
            if not TUNE["wp_late"]:
                nc.sync.dma_start(wp_sb[:],
                                  wp.rearrange("(mt p) n -> p mt n", p=P))

            a2a_in = dram.tile([M, ROWS_PC], f32)
            a2a_out = dram.tile([M, ROWS_PC], f32)

            def copy_cast(dst, src, which):
                if with_bias:
                    nc.scalar.activation(dst, src, AF.Identity,
                                         bias=bias_sb[:, which:which + 1])
                else:
                    nc.vector.tensor_copy(dst, src)

            if True:
                def emit_ph1(rb, ps1, acc_bufs, tp_bufs, xp, vp):
                    r0 = rb * RB
                    xt = xp.tile([P, NMT, RB], f32r, tag="xt", name="xt")
                    for mt in range(NMT):
                        nc.sync.dma_start(
                            xt[:, mt, :], xT[mt * P:(mt + 1) * P, r0:r0 + RB])
                    for which, (w_sb, dst) in enumerate(
                            ((wq_sb, QT), (wk_sb, KT), (wv_sb, None))):
                        acc = ps1.tile([P, RB], f32, tag="u", name="acc",
                                       bufs=acc_bufs)
                        for mt in range(NMT):
                            nc.tensor.matmul(acc[:], w_sb[:, mt, :], xt[:, mt, :],
                                             start=(mt == 0), stop=(mt == NMT - 1))
                        if dst is not None:
                            copy_cast(dst[:, r0:r0 + RB], acc[:], which)
                        else:
                            vt_sb = vp.tile([P, RB], f32r, tag="vt", name="vt_sb")
                            copy_cast(vt_sb[:], acc[:], which)
                            for k in range(RB // P):
                                t = rb * (RB // P) + k
                                tp = ps1.tile(
                                    [P, P], f32r, name="tp",
                                    tag="tp" if tp_bufs else "u",
                                    bufs=tp_bufs if tp_bufs else acc_bufs)
                                nc.tensor.transpose(
                                    tp[:], vt_sb[:, k * P:(k + 1) * P], ident[:])
                                nc.vector.tensor_copy(VA[:, t, 0:64], tp[:, 0:64])
                                nc.vector.tensor_copy(VA[:, t, 64:65],
                                                      c65_sb[:, 0:1])
                                nc.vector.tensor_copy(VB[:, t, 0:64],
                                                      c65_sb[:, 0:64])
                                nc.vector.tensor_copy(VB[:, t, 64:128],
                                                      tp[:, 64:128])

                def emit_ph2(b, qb, ps2, exp_pool, norm_pool):
                    gr0 = b * S + qb * QB
                    zt_a = ps2.tile([65, QB], f32, tag="zt", bufs=2, name="zt_a")
                    zt_b = ps2.tile([P, QB], f32, tag="zt", bufs=2, name="zt_b")
                    nkj = 4 * qb + 4
                    for t in range(nkj):
                        kj0 = 128 * t
                        di = t - 4 * qb
                        if di < 0:
                            col_off, w = 0, QB
                        elif di <= 1:
                            col_off, w = 128 * di, QB - 128 * di
                        else:
                            col_off, w = 256, 256
                        st2 = ps2.tile([P, 2 * QB], f32, tag="st2",
                                       bufs=TUNE["st2_bufs"], name="st2")
                        for h in range(2):
                            hp = slice(64 * h, 64 * h + 64)
                            nc.tensor.matmul(
                                st2[:, h * QB:h * QB + w],
                                KT[hp, b * S + kj0: b * S + kj0 + 128],
                                QT[hp, gr0 + col_off: gr0 + col_off + w],
                                start=True, stop=True)
                        ex = exp_pool.tile([P, 2, QB], f32r, tag="ex", name="ex")
                        st2v = st2.rearrange("p (h q) -> p h q", h=2)
                        nc.scalar.activation(ex[:, :, :w], st2v[:, :, :w], AF.Exp)
                        if di >= 0:
                            if di == 3:
                                nc.vector.tensor_tensor(
                                    ex[:, :, 0:256], ex[:, :, 0:256],
                                    tri2_sb[:, None, :].to_broadcast([P, 2, 256]),
                                    ALU.mult)
                            else:
                                nc.vector.tensor_tensor(
                                    ex[:, :, 0:128], ex[:, :, 0:128],
                                    tri1_sb[:, None, :].to_broadcast([P, 2, 128]),
                                    ALU.mult)
                        vt_idx = 16 * b + t
                        for h, (zt_x, vx) in enumerate(((zt_a, VA), (zt_b, VB))):
                            nc.tensor.matmul(
                                zt_x[:, col_off:col_off + w], vx[:, vt_idx, :],
                                ex[:, h, :w],
                                start=(t == 0), stop=(t == nkj - 1),
                                skip_group_check=True)
                    # normalize into ZT
                    recip = norm_pool.tile([P, QB], f32, tag="recip", name="recip")
                    nc.vector.reciprocal(recip[64:65, :], zt_a[64:65, :])
                    nc.vector.reciprocal(recip[0:1, :], zt_b[0:1, :])
                    rowa = norm_pool.tile([1, QB], f32, tag="rowa", name="rowa")
                    nc.sync.dma_start(rowa[:], recip[64:65, :])
                    bca = norm_pool.tile([64, QB], f32, tag="bca", name="bca")
                    bcb = norm_pool.tile([P, QB], f32, tag="bcb", name="bcb")
                    nc.gpsimd.partition_broadcast(bca[:], rowa[:], channels=64)
                    nc.gpsimd.partition_broadcast(bcb[:], recip[0:1, :],
                                                  channels=128)
                    nc.vector.tensor_tensor(ZT[0:64, gr0:gr0 + QB],
                                            zt_a[0:64, :], bca[:], ALU.mult)
                    nc.vector.tensor_tensor(ZT[64:128, gr0:gr0 + QB],
                                            zt_b[64:128, :], bcb[64:128, :],
                                            ALU.mult)
                    if phases >= 3:
                        j = gr0 // ROWS_PC
                        nc.sync.dma_start(
                            a2a_in[j * P:(j + 1) * P, :],
                            ZT.bitcast(f32)[:, gr0:gr0 + QB])

                def emit_iter(rep):
                    sfx = f"_{rep}"
                    with (
                        tc.tile_pool(name="xp" + sfx,
                                     bufs=TUNE["xp_bufs"]) as xp,
                        tc.tile_pool(name="vp" + sfx, bufs=2) as vp,
                        tc.tile_pool(name="ex" + sfx,
                                     bufs=TUNE["ex_bufs"]) as exp_pool,
                        tc.tile_pool(name="np" + sfx, bufs=2) as norm_pool,
                    ):
                        # rb0-3 with a wide PSUM pool (closes before ps2)
                        with tc.tile_pool(name="ps1a" + sfx, bufs=1,
                                          space="PSUM") as ps1a:
                            for rb in range(4):
                                emit_ph1(rb, ps1a, 4, 2, xp, vp)
                        # interleaved emission
                        with (
                            tc.tile_pool(name="ps1b" + sfx, bufs=1,
                                         space="PSUM") as ps1b,
                            tc.tile_pool(name="ps2" + sfx, bufs=1,
                                         space="PSUM") as ps2,
                        ):
                            for rb, blk in ((4, (0, 0)), (5, (0, 1)),
                                            (6, (0, 2)), (7, (0, 3))):
                                if phases >= 2:
                                    emit_ph2(*blk, ps2, exp_pool, norm_pool)
                                emit_ph1(rb, ps1b, TUNE["acc_bufs"], 0, xp, vp)
                            if TUNE["wp_late"] and rep == 0:
                                for mt in range(NMT):
                                    nc.sync.dma_start(
                                        wp_sb[:, mt, :],
                                        wp[mt * P:(mt + 1) * P, :])
                            if phases >= 2:
                                for qb in range(NQB):
                                    emit_ph2(1, qb, ps2, exp_pool, norm_pool)

                    # ---- all-to-all ----
                    if phases >= 3:
                        if for_sim:
                            nc.sync.dma_start(a2a_out[:], a2a_in[:])
                        else:
                            nc.gpsimd.collective_compute(
                                "AllToAll", ALU.bypass,
                                replica_groups=[list(range(NC))],
                                ins=[a2a_in.opt()], outs=[a2a_out.opt()],
                            )

                    # ---- phase 3: output projection ----
                    with (
                        tc.tile_pool(name="op" + sfx, bufs=2) as out_pool,
                        tc.tile_pool(name="ps3" + sfx, bufs=4,
                                     space="PSUM") as ps3,
                    ):
                        zt_sb = out_pool.tile([P, NMT, ROWS_PC], f32r, bufs=1,
                                              tag="zt_sb", name="zt_sb")
                        if phases >= 3:
                            for mt in range(NMT):
                                nc.sync.dma_start(
                                    zt_sb[:, mt, :],
                                    a2a_out.bitcast(f32r)[mt * P:(mt + 1) * P, :])
                        for rt in range(ROWS_PC // P if phases >= 3 else 0):
                            os_ = out_pool.tile([P, M], f32, tag="os", name="os_")
                            for nh in range(2):
                                acc = ps3.tile([P, 512], f32, tag="o",
                                               name="acc3")
                                for mt in range(NMT):
                                    nc.tensor.matmul(
                                        acc[:], zt_sb[:, mt, rt * P:(rt + 1) * P],
                                        wp_sb[:, mt, nh * 512:(nh + 1) * 512],
                                        start=(mt == 0), stop=(mt == NMT - 1))
                                nc.vector.tensor_copy(
                                    os_[:, nh * 512:(nh + 1) * 512], acc[:])
                            nc.sync.dma_start(out[rt * P:(rt + 1) * P, :], os_[:])

                for rep in range(repeat):
                    emit_iter(rep)

    nc.compile()
    _BUILD_CACHE[key] = nc
    return nc


def prep_inputs(x, W_attn, b_attn, W_proj, b_proj):
    x = np.asarray(x, dtype=np.float32)
    W_attn = np.asarray(W_attn, dtype=np.float32)
    b_attn = np.asarray(b_attn, dtype=np.float32)
    W_proj = np.asarray(W_proj, dtype=np.float32)

    xT = np.ascontiguousarray(x.reshape(R, M).T)
    tri1 = (np.arange(128)[None, :] >= np.arange(128)[:, None]).astype(np.float32)
    tri2 = (np.arange(256)[None, :] >= (np.arange(128) + 128)[:, None]).astype(
        np.float32)
    ident = np.eye(P, dtype=np.float32)
    c65 = np.zeros((P, 65), dtype=np.float32)
    c65[:, 0] = 1.0
    scale = 1.0 / np.sqrt(D)

    in_maps = []
    for c in range(NC):
        cs = slice(MC * c, MC * (c + 1))
        bq = b_attn[0 * M:1 * M][cs] * scale
        bk = b_attn[1 * M:2 * M][cs]
        bv = b_attn[2 * M:3 * M][cs]
        in_maps.append({
            "xT": xT,
            "wq": np.ascontiguousarray(W_attn[:, 0 * M:1 * M][:, cs] * scale),
            "wk": np.ascontiguousarray(W_attn[:, 1 * M:2 * M][:, cs]),
            "wv": np.ascontiguousarray(W_attn[:, 2 * M:3 * M][:, cs]),
            "bqkv": np.ascontiguousarray(np.stack([bq, bk, bv], axis=1)),
            "wp": W_proj,
            "tri1": tri1, "tri2": tri2, "ident_d": ident, "c65": c65,
        })
    return in_maps


def postprocess(results, b_proj):
    out = np.concatenate([results[c]["out"] for c in range(NC)], axis=0)
    out = out + np.asarray(b_proj, dtype=np.float32)[None, :]
    return out.reshape(B, S, M)


def kernel(x, W_attn, b_attn, W_proj, b_proj):
    nc = build_nc(with_bias=bool(np.any(np.asarray(b_attn))))
    in_maps = prep_inputs(x, W_attn, b_attn, W_proj, b_proj)
    res = run_bass_kernel_spmd(nc, in_maps, core_ids=list(range(NC)))
    return postprocess(res.results, b_proj)
